# revision 2
# baseline (speedup 1.0000x reference)
"""Trainium2 Bass kernel for nn_AttentiveRNNLanguageModel (8 NeuronCores, SPMD).

Full model on device per core (4 sequences/core, data-parallel over batch):
embedding-gather (host) -> gx GEMM -> fused LSTM(512) + positional LSTM(20)
recurrence -> gaussian attention + comb -> tied-embedding decoder GEMM.

HW exec time is measured by the replication-slope method: the NEFF body is
compiled at reps=1 and reps=3; exec_ns = (wall(reps3) - wall(reps1)) / 2 over
device-resident inputs with a persistent jit, which cancels the constant
axon dispatch overhead (~80ms) exactly.
"""

import sys, time

for _p in ("/opt/trn_rl_repo",):
    if _p not in sys.path:
        sys.path.insert(0, _p)

import numpy as np
import ml_dtypes
import jax
from jax.sharding import Mesh, PartitionSpec, NamedSharding
from jax.experimental.shard_map import shard_map

BF = ml_dtypes.bfloat16
N_CORES = 8

_COMPILED = {}

from contextlib import ExitStack

from concourse import bacc, tile, masks
import concourse.bass as bass
import concourse.mybir as mybir

F32 = mybir.dt.float32
BF16 = mybir.dt.bfloat16
AF = mybir.ActivationFunctionType
ALU = mybir.AluOpType
ts = bass.ts

V, B, T, H, P, NB = 10000, 32, 512, 512, 20, 3
BPC = 4          # sequences per core
G = 4 * H        # 2048 LSTM1 gates
GP = 4 * P       # 80 LSTM2 gates
KT = H // 128    # 4 k-tiles over H


def build_nc(t_steps=T, debug=False, reps=1, stages="ABCD", ablate=()):
    TT = t_steps
    KTT = TT // 128          # t-tiles in attention
    assert TT % 128 == 0

    nc = bacc.Bacc(None, target_bir_lowering=False, debug=False)

    # ---- inputs ----
    xT = nc.declare_dram_parameter("xT", [128, KT, BPC * TT], BF16, isOutput=False)
    WihT = nc.declare_dram_parameter("WihT", [128, KT, G], BF16, isOutput=False)
    WhhT = nc.declare_dram_parameter("WhhT", [128, KT, G], BF16, isOutput=False)
    gxbias = nc.declare_dram_parameter("gxbias", [1, G], F32, isOutput=False)
    WpihT = nc.declare_dram_parameter("WpihT", [128, KT, GP], BF16, isOutput=False)
    WphhT = nc.declare_dram_parameter("WphhT", [P, GP], BF16, isOutput=False)
    bp = nc.declare_dram_parameter("bp", [1, GP], BF16, isOutput=False)
    WheadT = nc.declare_dram_parameter("WheadT", [P, 4], BF16, isOutput=False)
    bhead = nc.declare_dram_parameter("bhead", [1, 4], BF16, isOutput=False)
    WcatT = nc.declare_dram_parameter("WcatT", [128, 2 * KT, H], BF16, isOutput=False)
    bcat = nc.declare_dram_parameter("bcat", [128, KT], F32, isOutput=False)
    embT = nc.declare_dram_parameter("embT", [128, KT, V], BF16, isOutput=False)
    recipj = nc.declare_dram_parameter("recipj", [1, TT], F32, isOutput=False)
    ramp = nc.declare_dram_parameter("ramp", [1, TT], F32, isOutput=False)
    tvals = nc.declare_dram_parameter("tvals", [128, KT], F32, isOutput=False)

    out = nc.declare_dram_parameter("out", [BPC * TT, V], BF16, isOutput=True)
    if debug:
        dbg_enc = nc.declare_dram_parameter("dbg_enc", [TT, BPC, H], BF16, isOutput=True)
        dbg_att = nc.declare_dram_parameter("dbg_att", [BPC, 4, TT], F32, isOutput=True)

    # ---- DRAM scratch ----
    gx_dram = nc.dram_tensor("gx_dram", [BPC, TT, G], BF16, kind="Internal")
    enc_dram = nc.dram_tensor("enc_dram", [TT, BPC, H], BF16, kind="Internal")
    att_dram = nc.dram_tensor("att_dram", [2, BPC, TT], F32, kind="Internal")
    rn_dram = nc.dram_tensor("rn_dram", [BPC, TT], F32, kind="Internal")

    with tile.TileContext(nc) as tc, ExitStack() as top:
        persist = top.enter_context(tc.tile_pool(name="persist", bufs=1))

        # resident weights
        whh_sb = persist.tile([128, KT, G], BF16)
        nc.sync.dma_start(out=whh_sb, in_=WhhT[:])
        wpih_sb = persist.tile([128, KT, GP], BF16)
        nc.sync.dma_start(out=wpih_sb, in_=WpihT[:])
        wphh_sb = persist.tile([P, GP], BF16)
        nc.sync.dma_start(out=wphh_sb, in_=WphhT[:])
        bp_sb = persist.tile([1, GP], BF16)
        nc.sync.dma_start(out=bp_sb, in_=bp[:])
        whead_sb = persist.tile([P, 4], BF16)
        nc.sync.dma_start(out=whead_sb, in_=WheadT[:])
        bhead_sb = persist.tile([1, 4], BF16)
        nc.sync.dma_start(out=bhead_sb, in_=bhead[:])
        wcat_sb = persist.tile([128, 2 * KT, H], BF16)
        nc.sync.dma_start(out=wcat_sb, in_=WcatT[:])
        bcat_sb = persist.tile([128, KT], F32)
        nc.sync.dma_start(out=bcat_sb, in_=bcat[:])
        recipj_bc = persist.tile([128, TT], F32)
        nc.sync.dma_start(out=recipj_bc, in_=recipj[:].broadcast_to([128, TT]))
        ramp_bc = persist.tile([BPC, TT], F32)
        nc.sync.dma_start(out=ramp_bc, in_=ramp[:].broadcast_to([BPC, TT]))
        tvals_sb = persist.tile([128, KT], F32)
        nc.sync.dma_start(out=tvals_sb, in_=tvals[:])
        gxbias_bc = persist.tile([128, G], F32)
        nc.sync.dma_start(out=gxbias_bc, in_=gxbias[:].broadcast_to([128, G]))

        ident = persist.tile([128, 128], BF16)
        masks.make_identity(nc, ident[:])
        ones_sb = persist.tile([128, 8], BF16)
        nc.vector.memset(ones_sb, 1.0)

        # persistent recurrence state
        c_sb = persist.tile([BPC, H], F32)
        hT_sb = persist.tile([128, KT, BPC], BF16)
        cp_sb = persist.tile([BPC, P], F32)
        hpT_sb = persist.tile([32, BPC], BF16)
        muw_sb = persist.tile([BPC, TT, 4], F32)
        mu_sb = persist.tile([BPC, TT], F32)
        s_sb = persist.tile([BPC, TT], F32)
        combT_all = persist.tile([128, BPC, KT, TT], BF16)

        def body(rep):
            nc.vector.memset(c_sb, 0.0)
            nc.vector.memset(hT_sb, 0.0)
            nc.vector.memset(cp_sb, 0.0)
            nc.vector.memset(hpT_sb, 0.0)

            # ---------------- Stage A: gx GEMM ----------------
            if "A" not in stages:
                return
            with ExitStack() as sa:
                a_in = sa.enter_context(tc.tile_pool(name=f"a_in{rep}", bufs=1))
                a_ps = sa.enter_context(tc.tile_pool(name=f"a_ps{rep}", bufs=4, space="PSUM"))
                a_gx = sa.enter_context(tc.tile_pool(name=f"a_gx{rep}", bufs=3))
                xt_sb = a_in.tile([128, KT, BPC * TT], BF16)
                nc.sync.dma_start(out=xt_sb, in_=xT[:])
                wih_sb = a_in.tile([128, KT, G], BF16)
                nc.sync.dma_start(out=wih_sb, in_=WihT[:])
                tpb = TT // 128  # token blocks per sequence
                for m in range(BPC * tpb):
                    gxa = a_gx.tile([128, G], BF16, tag="gxa")
                    for n in range(G // 512):
                        ps = a_ps.tile([128, 512], F32, tag="aps")
                        for k in range(KT):
                            nc.tensor.matmul(
                                ps, xt_sb[:, k, ts(m, 128)], wih_sb[:, k, ts(n, 512)],
                                start=(k == 0), stop=(k == KT - 1),
                            )
                        nc.vector.tensor_tensor(
                            out=gxa[:, ts(n, 512)], in0=ps,
                            in1=gxbias_bc[:, ts(n, 512)],
                            op=ALU.add,
                        )
                    b_i, tblk = m // tpb, m % tpb
                    nc.sync.dma_start(out=gx_dram[b_i, ts(tblk, 128), :], in_=gxa)

            # ---------------- Stage B: recurrence ----------------
            if "B" not in stages:
                return
            with ExitStack() as sb:
                gx_pool = sb.enter_context(tc.tile_pool(name=f"b_gx{rep}", bufs=3))
                gps_pool = sb.enter_context(tc.tile_pool(name=f"b_gps{rep}", bufs=1, space="PSUM"))
                hps_pool = sb.enter_context(tc.tile_pool(name=f"b_hps{rep}", bufs=1, space="PSUM"))
                pps_pool = sb.enter_context(tc.tile_pool(name=f"b_pps{rep}", bufs=1, space="PSUM"))
                ppt_pool = sb.enter_context(tc.tile_pool(name=f"b_ppt{rep}", bufs=1, space="PSUM"))
                hd_pool = sb.enter_context(tc.tile_pool(name=f"b_hd{rep}", bufs=1, space="PSUM"))
                tmp_pool = sb.enter_context(tc.tile_pool(name=f"b_tmp{rep}", bufs=2))
                h_pool = sb.enter_context(tc.tile_pool(name=f"b_h{rep}", bufs=3))

                for t in range(TT):
                    gx_t = gx_pool.tile([BPC, G], BF16, tag="gx")
                    nc.sync.dma_start(out=gx_t, in_=gx_dram[:, t, :])

                    # gate order [g | i | f | o] (host-permuted weights)
                    ps_gi = gps_pool.tile([BPC, 1024], F32, tag="pgi")
                    ps_fo = gps_pool.tile([BPC, 1024], F32, tag="pfo")
                    for n in range(4):
                        ps = ps_gi if n < 2 else ps_fo
                        dst = ps[:, ts(n % 2, 512)]
                        nc.tensor.matmul(
                            dst, ident[:BPC, :BPC], gx_t[:, ts(n, 512)],
                            start=True, stop=False,
                        )
                        for k in range(KT):
                            nc.tensor.matmul(
                                dst, hT_sb[:, k, :], whh_sb[:, k, ts(n, 512)],
                                start=False, stop=(k == KT - 1),
                            )

                    tanh_g = tmp_pool.tile([BPC, 512], F32, tag="tg")
                    nc.scalar.activation(tanh_g, ps_gi[:, 0:512], AF.Tanh)
                    sig_i = tmp_pool.tile([BPC, 512], F32, tag="si")
                    nc.scalar.activation(sig_i, ps_gi[:, 512:1024], AF.Sigmoid)
                    sig_f = tmp_pool.tile([BPC, 512], F32, tag="sf")
                    nc.scalar.activation(sig_f, ps_fo[:, 0:512], AF.Sigmoid)
                    sig_o = tmp_pool.tile([BPC, 512], F32, tag="so")
                    nc.scalar.activation(sig_o, ps_fo[:, 512:1024], AF.Sigmoid)

                    t1 = tmp_pool.tile([BPC, 512], F32, tag="t1")
                    nc.vector.tensor_tensor(out=t1, in0=sig_i, in1=tanh_g, op=ALU.mult)
                    t2 = tmp_pool.tile([BPC, 512], F32, tag="t2")
                    nc.vector.tensor_tensor(out=t2, in0=sig_f, in1=c_sb, op=ALU.mult)
                    nc.vector.tensor_tensor(out=c_sb, in0=t1, in1=t2, op=ALU.add)
                    th_c = tmp_pool.tile([BPC, 512], F32, tag="thc")
                    nc.scalar.activation(th_c, c_sb, AF.Tanh)
                    h_bf = h_pool.tile([BPC, H], BF16, tag="h")
                    nc.vector.tensor_tensor(out=h_bf, in0=sig_o, in1=th_c, op=ALU.mult)

                    # h -> hT (PE transpose, packed into one psum tile)
                    hps = hps_pool.tile([128, KT, BPC], BF16, tag="hps")
                    for k in range(KT):
                        nc.tensor.transpose(hps[:, k, :], h_bf[:, ts(k, 128)], ident[:BPC, :BPC])
                    nc.vector.tensor_copy(out=hT_sb, in_=hps)

                    nc.sync.dma_start(out=enc_dram[t], in_=h_bf)

                    # ---- LSTM2 ----
                    ps_p = pps_pool.tile([BPC, GP], F32, tag="pgp")
                    nc.tensor.matmul(ps_p, ones_sb[0:1, 0:BPC], bp_sb, start=True, stop=False)
                    nc.tensor.matmul(ps_p, hpT_sb[:P, :], wphh_sb, start=False, stop=False)
                    for k in range(KT):
                        nc.tensor.matmul(
                            ps_p, hT_sb[:, k, :], wpih_sb[:, k, :],
                            start=False, stop=(k == KT - 1),
                        )
                    tanh_gp = tmp_pool.tile([BPC, P], F32, tag="tgp")
                    nc.scalar.activation(tanh_gp, ps_p[:, 0:P], AF.Tanh)
                    sig_ifop = tmp_pool.tile([BPC, 3 * P], F32, tag="sifop")
                    nc.scalar.activation(sig_ifop, ps_p[:, P : 4 * P], AF.Sigmoid)
                    tp1 = tmp_pool.tile([BPC, P], F32, tag="tp1")
                    nc.vector.tensor_tensor(out=tp1, in0=sig_ifop[:, 0:P], in1=tanh_gp, op=ALU.mult)
                    tp2 = tmp_pool.tile([BPC, P], F32, tag="tp2")
                    nc.vector.tensor_tensor(out=tp2, in0=sig_ifop[:, P : 2 * P], in1=cp_sb, op=ALU.mult)
                    nc.vector.tensor_tensor(out=cp_sb, in0=tp1, in1=tp2, op=ALU.add)
                    th_cp = tmp_pool.tile([BPC, P], F32, tag="thcp")
                    nc.scalar.activation(th_cp, cp_sb, AF.Tanh)
                    hp_bf = tmp_pool.tile([BPC, P], BF16, tag="hp")
                    nc.vector.tensor_tensor(out=hp_bf, in0=sig_ifop[:, 2 * P : 3 * P], in1=th_cp, op=ALU.mult)
                    ppt = ppt_pool.tile([P, BPC], BF16, tag="ppt")
                    nc.tensor.transpose(ppt, hp_bf, ident[:BPC, :BPC])
                    nc.vector.tensor_copy(out=hpT_sb[:P, :], in_=ppt)

                    # mu/sig head (reads hpT of this step)
                    hd = hd_pool.tile([BPC, 4], F32, tag="hd")
                    nc.tensor.matmul(hd, ones_sb[0:1, 0:BPC], bhead_sb, start=True, stop=False)
                    nc.tensor.matmul(hd, hpT_sb[:P, :], whead_sb, start=False, stop=True)
                    nc.vector.tensor_copy(out=muw_sb[:, t, :], in_=hd)

            # ---------------- post-loop: mu scan, sigma prep ----------------
            with ExitStack() as sp:
                pp = sp.enter_context(tc.tile_pool(name=f"post{rep}", bufs=1))
                nc.scalar.activation(muw_sb[:, :, 0:3], muw_sb[:, :, 0:3], AF.Relu)
                nc.scalar.activation(muw_sb[:, :, 3:4], muw_sb[:, :, 3:4], AF.Sigmoid)
                d1 = pp.tile([BPC, TT], F32)
                nc.vector.tensor_scalar(
                    out=d1, in0=muw_sb[:, :, 1], scalar1=1.0 / TT, scalar2=None, op0=ALU.mult
                )
                d2 = pp.tile([BPC, TT], F32)
                nc.vector.tensor_tensor(
                    out=d2, in0=muw_sb[:, :, 2],
                    in1=ramp_bc, op=ALU.mult,
                )
                nc.vector.tensor_tensor(out=d1, in0=d1, in1=d2, op=ALU.add)
                nc.vector.tensor_tensor_scan(
                    out=mu_sb, data0=muw_sb[:, :, 0], data1=d1,
                    initial=0.0, op0=ALU.mult, op1=ALU.add,
                )
                # s = -1/(2 sig^2)
                sg2 = pp.tile([BPC, TT], F32)
                nc.vector.tensor_tensor(
                    out=sg2, in0=muw_sb[:, :, 3], in1=muw_sb[:, :, 3], op=ALU.mult
                )
                nc.vector.tensor_scalar(
                    out=sg2, in0=sg2, scalar1=2.0, scalar2=None, op0=ALU.mult
                )
                nc.vector.reciprocal(out=s_sb, in_=sg2)
                nc.vector.tensor_scalar(
                    out=s_sb, in0=s_sb, scalar1=-1.0, scalar2=None, op0=ALU.mult
                )
                nc.sync.dma_start(out=att_dram[0], in_=mu_sb)
                nc.sync.dma_start(out=att_dram[1], in_=s_sb)
                if debug and rep == 0:
                    nc.sync.dma_start(out=dbg_att[:, 0, :], in_=mu_sb)
                    nc.sync.dma_start(out=dbg_att[:, 1, :], in_=muw_sb[:, :, 3])
                    nc.sync.dma_start(out=dbg_att[:, 2, :], in_=s_sb)

            # ---------------- Stage C1: attention + comb ----------------
            if "C" not in stages:
                return
            with ExitStack() as sc:
                enc_pool = sc.enter_context(tc.tile_pool(name=f"c_enc{rep}", bufs=2))
                w_pool = sc.enter_context(tc.tile_pool(name=f"c_w{rep}", bufs=2))
                ctmp_pool = sc.enter_context(tc.tile_pool(name=f"c_tmp{rep}", bufs=2))
                nps_pool = sc.enter_context(tc.tile_pool(name=f"c_nps{rep}", bufs=2, space="PSUM"))
                tps_pool = sc.enter_context(tc.tile_pool(name=f"c_tps{rep}", bufs=2, space="PSUM"))
                cps_pool = sc.enter_context(tc.tile_pool(name=f"c_cps{rep}", bufs=2, space="PSUM"))
                cat_pool = sc.enter_context(tc.tile_pool(name=f"c_cat{rep}", bufs=2))
                nrm_pool = sc.enter_context(tc.tile_pool(name=f"c_nrm{rep}", bufs=2))

                for b in range(BPC):
                    mu_bc = ctmp_pool.tile([128, TT], F32, tag="mubc", bufs=1)
                    nc.sync.dma_start(
                        out=mu_bc, in_=att_dram[0, b : b + 1, :].broadcast_to([128, TT])
                    )
                    s_bc = ctmp_pool.tile([128, TT], F32, tag="sbc", bufs=1)
                    nc.sync.dma_start(
                        out=s_bc, in_=att_dram[1, b : b + 1, :].broadcast_to([128, TT])
                    )
                    enc_sb = enc_pool.tile([128, KTT, H], BF16, tag="enc")
                    for k in range(KTT):
                        nc.sync.dma_start(out=enc_sb[:, k, :], in_=enc_dram[ts(k, 128), b, :])

                    wT = w_pool.tile([128, KTT, TT], BF16, tag="wt")
                    nps = nps_pool.tile([1, TT], F32, tag="nrm")
                    for k in range(KTT):
                        dt_ = ctmp_pool.tile([128, TT], F32, tag="dt")
                        nc.vector.scalar_tensor_tensor(
                            out=dt_,
                            in0=recipj_bc,
                            scalar=tvals_sb[:, k : k + 1],
                            in1=mu_bc,
                            op0=ALU.mult, op1=ALU.subtract,
                        )
                        nc.vector.tensor_tensor(out=dt_, in0=dt_, in1=dt_, op=ALU.mult)
                        nc.vector.tensor_tensor(
                            out=dt_, in0=dt_,
                            in1=s_bc, op=ALU.mult,
                        )
                        we = ctmp_pool.tile([128, TT], F32, tag="we")
                        nc.scalar.activation(we, dt_, AF.Exp)
                        nc.gpsimd.affine_select(
                            out=wT[:, k, :], in_=we, pattern=[[1, TT]],
                            compare_op=ALU.is_ge, fill=0.0,
                            base=-128 * k, channel_multiplier=-1,
                        )
                        w2 = w_pool.tile([128, TT], BF16, tag="w2", bufs=2)
                        nc.vector.tensor_tensor(
                            out=w2, in0=wT[:, k, :], in1=wT[:, k, :], op=ALU.mult
                        )
                        nc.tensor.matmul(
                            nps, ones_sb[:, 0:1], w2,
                            start=(k == 0), stop=(k == KTT - 1),
                        )
                    # rnorm = 1/sqrt(max(n2,eps))
                    rn = nrm_pool.tile([1, TT], F32, tag="rn")
                    nc.vector.tensor_scalar(
                        out=rn, in0=nps, scalar1=1e-24, scalar2=None, op0=ALU.max
                    )
                    nc.vector.reciprocal(out=rn, in_=rn)
                    nc.scalar.activation(rn, rn, AF.Sqrt)
                    nc.sync.dma_start(out=rn_dram[b : b + 1, :], in_=rn)
                    rn_bc = ctmp_pool.tile([128, TT], F32, tag="rnbc", bufs=1)
                    nc.sync.dma_start(
                        out=rn_bc, in_=rn_dram[b : b + 1, :].broadcast_to([128, TT])
                    )
                    if debug and rep == 0:
                        nc.sync.dma_start(out=dbg_att[b : b + 1, 3, :], in_=rn)

                    cat_sb = cat_pool.tile([128, 2 * KT, TT], BF16, tag="cat")
                    # ctx^T = enc(stationary) x wT(moving), scaled by rnorm
                    for m in range(KT):
                        cps = cps_pool.tile([128, TT], F32, tag="ctx")
                        for k in range(KTT):
                            nc.tensor.matmul(
                                cps, enc_sb[:, k, ts(m, 128)], wT[:, k, :],
                                start=(k == 0), stop=(k == KTT - 1),
                            )
                        nc.vector.tensor_tensor(
                            out=cat_sb[:, m, :], in0=cps,
                            in1=rn_bc, op=ALU.mult,
                        )
                    # enc^T via PE transpose
                    for k in range(KTT):
                        for m in range(KT):
                            tps = tps_pool.tile([128, 128], BF16, tag="tp")
                            nc.tensor.transpose(tps, enc_sb[:, k, ts(m, 128)], ident)
                            nc.any.tensor_copy(out=cat_sb[:, KT + m, ts(k, 128)], in_=tps)
                    # comb^T = tanh(WcatT^T x cat + bcat)
                    for m in range(KT):
                        cps = cps_pool.tile([128, TT], F32, tag="comb")
                        for k in range(2 * KT):
                            nc.tensor.matmul(
                                cps, wcat_sb[:, k, ts(m, 128)], cat_sb[:, k, :],
                                start=(k == 0), stop=(k == 2 * KT - 1),
                            )
                        nc.scalar.activation(
                            combT_all[:, b, m, :], cps, AF.Tanh,
                            bias=bcat_sb[:, m : m + 1],
                        )

            # ---------------- Stage C2: decoder ----------------
            if "D" not in stages:
                return
            NV = 500
            NVG = V // NV  # 20
            with ExitStack() as sd:
                d_w = sd.enter_context(tc.tile_pool(name=f"d_w{rep}", bufs=1))
                dps_pool = sd.enter_context(tc.tile_pool(name=f"d_ps{rep}", bufs=6, space="PSUM"))
                do_pool = sd.enter_context(tc.tile_pool(name=f"d_out{rep}", bufs=2))
                embt_sb = d_w.tile([128, KT, V], BF16)
                nc.sync.dma_start(out=embt_sb, in_=embT[:])
                for b in range(BPC):
                    for jm in range(KTT):
                        for vh in range(2):
                            orow = do_pool.tile([128, V // 2], BF16, tag="orow")
                            for vg_ in range(NVG // 2):
                                vg = vh * (NVG // 2) + vg_
                                dps = dps_pool.tile([128, NV], F32, tag="dps")
                                for k in range(KT):
                                    nc.tensor.matmul(
                                        dps,
                                        combT_all[:, b, k, ts(jm, 128)],
                                        embt_sb[:, k, ts(vg, NV)],
                                        start=(k == 0), stop=(k == KT - 1),
                                    )
                                nc.any.tensor_copy(out=orow[:, ts(vg_, NV)], in_=dps)
                            nc.sync.dma_start(
                                out=out[
                                    b * TT + 128 * jm : b * TT + 128 * (jm + 1),
                                    vh * (V // 2) : (vh + 1) * (V // 2),
                                ],
                                in_=orow,
                            )

            if debug and rep == 0:
                nc.sync.dma_start(out=dbg_enc[:], in_=enc_dram[:])

        for rep in range(reps):
            body(rep)

    nc.compile()
    return nc




def _perm_gates(a, hc):
    """[4*hc, ...] torch gate order i,f,g,o -> device order g,i,f,o along axis 0."""
    i, f, g, o = a[0:hc], a[hc:2*hc], a[2*hc:3*hc], a[3*hc:4*hc]
    import numpy as _np
    return _np.concatenate([g, i, f, o], axis=0)


def _kxm(a):
    """[K, M] -> [128, K//128, M] with K = k*128 + p."""
    K, M = a.shape
    return np.ascontiguousarray(a.reshape(K // 128, 128, M).transpose(1, 0, 2))


def prep_inputs(inputs, t_steps=T):
    TT = t_steps
    f32 = np.float32
    embedding = np.asarray(inputs["embedding"], f32)
    idx = np.asarray(inputs["input"]).astype(np.int64)[:, :TT]
    Wih = np.asarray(inputs["W_ih"], f32)
    Whh = np.asarray(inputs["W_hh"], f32)
    gxbias = (np.asarray(inputs["b_ih"], f32) + np.asarray(inputs["b_hh"], f32))[None, :]
    Wpih = np.asarray(inputs["Wp_ih"], f32)
    Wphh = np.asarray(inputs["Wp_hh"], f32)
    bp = (np.asarray(inputs["bp_ih"], f32) + np.asarray(inputs["bp_hh"], f32))[None, :]
    Wmu = np.asarray(inputs["W_mu"], f32)
    bmu = np.asarray(inputs["b_mu"], f32)
    Wsig = np.asarray(inputs["W_sig"], f32)
    bsig = np.asarray(inputs["b_sig"], f32)
    Wcat = np.asarray(inputs["W_cat"], f32)
    bcat = np.asarray(inputs["b_cat"], f32)

    shared = {
        "WihT": _kxm(_perm_gates(Wih, H).T).astype(BF),
        "WhhT": _kxm(_perm_gates(Whh, H).T).astype(BF),
        "gxbias": _perm_gates(gxbias[0], H)[None, :],
        "WpihT": _kxm(_perm_gates(Wpih, P).T).astype(BF),
        "WphhT": np.ascontiguousarray(_perm_gates(Wphh, P).T).astype(BF),
        "bp": _perm_gates(bp[0], P)[None, :].astype(BF),
        "WheadT": np.concatenate([Wmu.T, Wsig.T], axis=1).astype(BF),
        "bhead": np.concatenate([bmu, bsig])[None, :].astype(BF),
        "WcatT": _kxm(Wcat.T).astype(BF),
        "bcat": np.ascontiguousarray(bcat.reshape(KT, 128).T).astype(f32),
        "embT": _kxm(embedding.T).astype(BF),
        "recipj": (1.0 / (np.arange(TT, dtype=f32) + 1.0))[None, :],
        "ramp": ((np.arange(TT, dtype=f32) + 1.0) / TT)[None, :],
        "tvals": np.ascontiguousarray(
            (np.arange(H, dtype=f32).reshape(KT, 128).T)
        ),
    }
    in_maps = []
    for i in range(N_CORES):
        own = idx[i * BPC : (i + 1) * BPC]          # [4, TT]
        toks = embedding[own.reshape(-1)]           # [4*TT, H]
        m = dict(shared)
        m["xT"] = _kxm(np.ascontiguousarray(toks.T)).astype(BF)
        in_maps.append(m)
    return in_maps


def assemble(results, t_steps=T):
    TT = t_steps
    full = np.empty((B, TT, V), np.float32)
    for i in range(N_CORES):
        o = np.asarray(results[i]["out"], dtype=np.float32)  # [BPC*TT, V]
        for b in range(BPC):
            full[i * BPC + b] = o[b * TT : (b + 1) * TT]
    return full


class SpmdRunner:
    def __init__(self, nc, n_cores=8):
        from concourse.bass2jax import install_neuronx_cc_hook
        install_neuronx_cc_hook()
        self.nc = nc
        self.n_cores = n_cores
        assert nc.dbg_addr is None or not nc.dbg_callbacks
        partition_name = (
            nc.partition_id_tensor.name if nc.partition_id_tensor else None
        )
        in_names, out_names, out_avals, zero_outs = [], [], [], []
        for alloc in nc.m.functions[0].allocations:
            if not isinstance(alloc, mybir.MemoryLocationSet):
                continue
            name = alloc.memorylocations[0].name
            if alloc.kind == "ExternalInput":
                if name != partition_name and name != (
                    nc.dbg_addr.name if nc.dbg_addr else None
                ):
                    in_names.append(name)
            elif alloc.kind == "ExternalOutput":
                shape = tuple(alloc.tensor_shape)
                dtype = mybir.dt.np(alloc.dtype)
                out_names.append(name)
                out_avals.append(jax.core.ShapedArray(shape, dtype))
                zero_outs.append(np.zeros(shape, dtype))
        self.in_names, self.out_names = in_names, out_names
        self.out_avals, self.zero_outs = out_avals, zero_outs
        self.partition_name = partition_name
        n_params, n_outs = len(in_names), len(out_names)
        self.n_params, self.n_outs = n_params, n_outs

        all_names = list(in_names) + list(out_names)
        if nc.dbg_addr is not None:
            all_names.append(nc.dbg_addr.name)
            self.dbg_zero = np.zeros((1, 2), np.uint32)
        else:
            self.dbg_zero = None
        if partition_name is not None:
            all_names.append(partition_name)

        from concourse.bass2jax import _bass_exec_p, partition_id_tensor

        def _body(*args):
            operands = list(args)
            if self.dbg_zero is not None:
                operands.append(
                    jax.numpy.broadcast_to(jax.numpy.zeros((1, 2), jax.numpy.uint32), (1, 2))
                )
            if partition_name is not None:
                operands.append(partition_id_tensor())
            outs = _bass_exec_p.bind(
                *operands,
                out_avals=tuple(out_avals),
                in_names=tuple(all_names),
                out_names=tuple(out_names),
                lowering_input_output_aliases=(),
                sim_require_finite=True,
                sim_require_nnan=True,
                nc=nc,
            )
            return tuple(outs)

        devices = jax.devices()[:n_cores]
        self.mesh = Mesh(np.asarray(devices), ("core",))
        in_specs = (PartitionSpec("core"),) * (n_params + n_outs)
        out_specs = (PartitionSpec("core"),) * n_outs
        self.fn = jax.jit(
            shard_map(_body, mesh=self.mesh, in_specs=in_specs,
                      out_specs=out_specs, check_rep=False),
            donate_argnums=tuple(range(n_params, n_params + n_outs)),
            keep_unused=True,
        )

    def upload_inputs(self, in_maps):
        sh = NamedSharding(self.mesh, PartitionSpec("core"))
        self.in_dev = []
        for i, name in enumerate(self.in_names):
            cat = np.concatenate(
                [np.asarray(in_maps[c][name]) for c in range(self.n_cores)], axis=0
            )
            self.in_dev.append(jax.device_put(cat, sh))

    def _zero_set(self):
        sh = NamedSharding(self.mesh, PartitionSpec("core"))
        return [
            jax.device_put(
                np.zeros((self.n_cores * z.shape[0], *z.shape[1:]), z.dtype), sh
            )
            for z in self.zero_outs
        ]

    def run(self, fetch=True):
        outs = self.fn(*self.in_dev, *self._zero_set())
        for o in outs:
            o.block_until_ready()
        if not fetch:
            return None
        res = []
        for c in range(self.n_cores):
            res.append({
                name: np.asarray(outs[i]).reshape(
                    self.n_cores, *self.out_avals[i].shape
                )[c]
                for i, name in enumerate(self.out_names)
            })
        return res

    def time_runs(self, n_iters=5):
        zero_sets = [self._zero_set() for _ in range(n_iters)]
        for zs in zero_sets:
            for z in zs:
                z.block_until_ready()
        times = []
        for it in range(n_iters):
            t0 = time.perf_counter()
            outs = self.fn(*self.in_dev, *zero_sets[it])
            for o in outs:
                o.block_until_ready()
            times.append(time.perf_counter() - t0)
            del outs
        return times


# ----------------------------------------------------------------------------
# top-level entry
# ----------------------------------------------------------------------------

def _run_and_time(in_maps):
    from concourse import bass_utils

    nc1 = build_nc(t_steps=T, debug=False, reps=1)
    res = bass_utils.run_bass_kernel_spmd(nc1, in_maps, list(range(N_CORES)))
    results = res.results

    exec_ns = None
    try:
        r1 = SpmdRunner(nc1, N_CORES)
        r1.upload_inputs(in_maps)
        r1.run(fetch=False)
        nc3 = build_nc(t_steps=T, debug=False, reps=3)
        r3 = SpmdRunner(nc3, N_CORES)
        r3.upload_inputs(in_maps)
        r3.run(fetch=False)
        w1 = min(r1.time_runs(6))
        w3 = min(r3.time_runs(6))
        exec_ns = max(int((w3 - w1) / 2 * 1e9), 1)
    except Exception:
        import traceback
        traceback.print_exc()
    if exec_ns is None or exec_ns <= 0:
        exec_ns = res.exec_time_ns or int(30e9)
    _COMPILED["exec_time_ns"] = exec_ns
    return results


def kernel(input, h0, c0, embedding, dec_bias, W_ih, W_hh, b_ih, b_hh,
           Wp_ih, Wp_hh, bp_ih, bp_hh, W_mu, b_mu, W_sig, b_sig, W_cat, b_cat):
    inputs = dict(
        input=input, h0=h0, c0=c0, embedding=embedding, dec_bias=dec_bias,
        W_ih=W_ih, W_hh=W_hh, b_ih=b_ih, b_hh=b_hh, Wp_ih=Wp_ih, Wp_hh=Wp_hh,
        bp_ih=bp_ih, bp_hh=bp_hh, W_mu=W_mu, b_mu=b_mu, W_sig=W_sig,
        b_sig=b_sig, W_cat=W_cat, b_cat=b_cat,
    )
    in_maps = prep_inputs(inputs, t_steps=T)
    results = _run_and_time(in_maps)
    decoded = assemble(results, t_steps=T)      # [B, T, V] fp32
    db = np.asarray(dec_bias, np.float32)
    if np.any(db):
        decoded = decoded + db
    return decoded
